# revision 1
# baseline (speedup 1.0000x reference)
"""Titans NeuralMemory forward on 8 Trainium2 NeuronCores.

Decomposition (validated vs reference in fp64/numpy):
  - Per-chunk MLP-loss gradients are rank-16: g_i(s) = l_i(s)^T r_i(s) with
    l/r factors [16, 256] from a batched forward/backward pass with the
    shared base weights.
  - The two associative scans have scalar per-chunk coefficients, so their
    composition is a lower-triangular [64, 64] matrix T = L_D @ L_A built
    stably via exp of cumulative log-sigmoid differences.
  - Retrieval never materializes fast weights: per layer,
      X_{i+1} = silu(X_i @ W_i + (X_i @ L_i^T * M) @ R_i),
    where M[r, j] = T[chunk(r), chunk(j)] expands T blockwise.

Sharding: 8 cores = 2 batch rows x 4 retrieve row-groups of 256 rows.
Each core redundantly runs the store phase for its batch row and computes
its own 256 retrieve rows; no collectives. Matmuls in fp32r (full PE rate).
"""
import os
import numpy as np

import concourse.bass as bass
import concourse.tile as tile
from concourse import bacc, mybir
from concourse.bass_utils import run_bass_kernel_spmd

AF = mybir.ActivationFunctionType
ALU = mybir.AluOpType
FP32 = mybir.dt.float32
FP32R = mybir.dt.float32r

B, L, D, C, DEPTH = 2, 1024, 256, 16, 4
N = L // C          # 64 chunks
P = 128
EPS = 1.1920929e-07
NCORES = 8
GROUPS = 4
RT = L // GROUPS    # 256 retrieve rows per core

# weight-blob layout (fp32r, per-partition fp32 word offsets)
WQ_O, WKV_O = 0, 512
W_O = WKV_O + 1024
WT_O = W_O + 2048
WP_O = WT_O + 1536
IDR_O = WP_O + 8
UT_O = IDR_O + 128
NUT_O = UT_O + 128
SEL_O = NUT_O + 128
WTS_SZ = SEL_O + 16

_CACHE = {}
LAST_PERF = {}


def _install_ntff_hook():
    """The agent image's antenv lacks axon_hooks; synthesize it so
    run_bass_kernel_spmd's trace=True path can reach the NTFF ctypes hook."""
    import sys
    import types
    try:
        from trn_agent_boot.trn_boot import _ntff_profile_via_ctypes
        hook = _ntff_profile_via_ctypes("/opt/axon/libaxon_pjrt.so")
    except Exception:
        return False
    if hook is None:
        return False
    mod = types.ModuleType("antenv.axon_hooks")
    mod.get_axon_ntff_profile_hook = lambda: hook
    mod.set_axon_ntff_profile_hook = lambda h: None
    sys.modules["antenv.axon_hooks"] = mod
    return True


def _build():
    nc = bacc.Bacc("TRN2", target_bir_lowering=False)

    seq_b = nc.dram_tensor("seq_b", [L, D], FP32, kind="ExternalInput")
    seq_q = nc.dram_tensor("seq_q", [RT, D], FP32, kind="ExternalInput")
    wts_d = nc.dram_tensor("wts_d", [P, WTS_SZ], FP32R, kind="ExternalInput")
    cst_d = nc.dram_tensor("cst_d", [P, 2 * N], FP32, kind="ExternalInput")
    out_d = nc.dram_tensor("out", [RT, D], FP32, kind="ExternalOutput")

    with tile.TileContext(nc) as tc:
        with (
            tc.tile_pool(name="big", bufs=1) as big,
            tc.tile_pool(name="rot", bufs=3) as rot,
            tc.tile_pool(name="pmm", bufs=2, space="PSUM") as pmm,
            tc.tile_pool(name="psc", bufs=2, space="PSUM") as psc,
            tc.tile_pool(name="ptr", bufs=2, space="PSUM") as ptr,
            tc.tile_pool(name="dram", bufs=1, space="DRAM") as dram,
        ):
            # ---------------- bulk loads ----------------
            wts = big.tile([P, WTS_SZ], FP32R)
            nc.sync.dma_start(wts, wts_d[:])
            cst = big.tile([P, 2 * N], FP32)
            nc.sync.dma_start(cst, cst_d[:])
            sq8 = big.tile([P, 8, D], FP32, tag="sq8")
            nc.sync.dma_start(sq8, seq_b[:].rearrange("(i p) d -> p i d", p=P))
            qs2 = big.tile([P, 2, D], FP32, tag="qs2")
            nc.sync.dma_start(qs2, seq_q[:].rearrange("(i p) d -> p i d", p=P))

            wq_sb = wts[:, WQ_O:WQ_O + 512].rearrange("p (k m) -> p k m", k=2)
            wkv_sb = wts[:, WKV_O:WKV_O + 1024].rearrange("p (k m) -> p k m", k=2)
            w_sb = wts[:, W_O:W_O + 2048].rearrange(
                "p (l k m) -> p l k m", l=4, k=2)
            wt_sb = wts[:, WT_O:WT_O + 1536].rearrange(
                "p (l k m) -> p l k m", l=3, k=2)
            wp_sb = wts[:, WP_O:WP_O + 8].rearrange("p (k m) -> p k m", k=2)
            identR = wts[:, IDR_O:IDR_O + 128]
            ut_sb = wts[:, UT_O:UT_O + 128]
            nut_sb = wts[:, NUT_O:NUT_O + 128]
            sel_sb = wts[:, SEL_O:SEL_O + 16]
            mls_sb = cst[:, 0:N]
            mut_sb = cst[:, N:2 * N]

            eps_sb = big.tile([P, 1], FP32)
            nc.vector.memset(eps_sb, EPS)

            # ---------------- rmsnorms (store + retrieve-q) ----------------
            def rmsnorm_make(x, tag):
                scr_a = rot.tile([P, D], FP32, tag="rms_scr", bufs=2)
                ms = rot.tile([P, 1], FP32, tag=f"{tag}ms", bufs=2)
                nc.scalar.activation(scr_a, x, AF.Square, accum_out=ms)
                lnv = rot.tile([P, 1], FP32, tag=f"{tag}ln", bufs=2)
                nc.scalar.activation(lnv, ms, AF.Ln, scale=1.0 / D, bias=eps_sb)
                rstd = rot.tile([P, 1], FP32, tag=f"{tag}rs", bufs=2)
                nc.scalar.activation(rstd, lnv, AF.Exp, scale=-0.5)
                out = rot.tile([P, D], FP32R, tag=f"{tag}o", bufs=4 if tag == "sn" else 2)
                nc.vector.tensor_scalar_mul(out, x, rstd)
                return out

            sn = [rmsnorm_make(sq8[:, i, :], "sn") for i in range(8)]
            rq = [rmsnorm_make(qs2[:, i, :], "rq") for i in range(2)]

            # ---------------- transposes: snT, rqT ----------------
            snT = [big.tile([P, L], FP32R, name=f"snT{k}", tag=f"snT{k}")
                   for k in range(2)]
            for grp in range(2):
                for ko in range(2):
                    tp = ptr.tile([P, 512], FP32R, tag="tr")
                    for ii in range(4):
                        i = grp * 4 + ii
                        nc.tensor.transpose(
                            tp[:, ii * P:(ii + 1) * P],
                            sn[i][:, ko * P:(ko + 1) * P], identR)
                    nc.vector.tensor_copy(
                        snT[ko][:, grp * 512:(grp + 1) * 512], tp)
            rqT = [big.tile([P, RT], FP32R, name=f"rqT{k}") for k in range(2)]
            for ko in range(2):
                tp = ptr.tile([P, 512], FP32R, tag="tr")
                for rt in range(2):
                    nc.tensor.transpose(
                        tp[:, rt * P:(rt + 1) * P],
                        rq[rt][:, ko * P:(ko + 1) * P], identR)
                nc.vector.tensor_copy(rqT[ko], tp[:, 0:RT])

            # ---------------- chunk sums -> T pipeline ----------------
            cmT = big.tile([P, 2, N], FP32R)
            with nc.allow_low_precision(reason="fp32r rounding of fp32 accum"):
                for ko in range(2):
                    nc.vector.reduce_sum(
                        cmT[:, ko, :],
                        snT[ko].rearrange("p (n c) -> p n c", c=C),
                        axis=mybir.AxisListType.X)

            zp = ptr.tile([N, 4], FP32, tag="tr")
            for ko in range(2):
                nc.tensor.matmul(zp, cmT[:, ko, :], wp_sb[:, ko, :],
                                 start=(ko == 0), stop=(ko == 1))
            # sigmoids first (one table), then ln/exp cluster
            sg = big.tile([P, 3], FP32)
            nc.vector.memset(sg, 0.0)
            nc.scalar.activation(sg[:N, 0:1], zp[:, 1:2], AF.Sigmoid)
            nc.scalar.activation(sg[:N, 1:2], zp[:, 2:3], AF.Sigmoid, scale=-1.0)
            nc.scalar.activation(sg[:N, 2:3], zp[:, 0:1], AF.Sigmoid)
            lg = big.tile([P, 3], FP32)
            nc.vector.memset(lg, 0.0)
            nc.scalar.activation(lg[:N, :], sg[:N, :], AF.Ln)
            lgr = big.tile([P, 2], FP32R)
            nc.vector.tensor_copy(lgr, lg[:, 0:2])
            cacc_p = ptr.tile([P, 2], FP32, tag="tr")
            nc.tensor.matmul(cacc_p, ut_sb, lgr, start=True, stop=True)
            cacc = big.tile([P, 2], FP32)
            nc.vector.tensor_copy(cacc, cacc_p)
            nacc_p = ptr.tile([P, 2], FP32, tag="tr")
            nc.tensor.matmul(nacc_p, nut_sb, lgr, start=True, stop=True)
            nacc = big.tile([P, 2], FP32)
            nc.vector.tensor_copy(nacc, nacc_p)

            # stage [NACC0 + ln(2 lr / D) | CACC1] -> DRAM -> row-bcasts.
            # Folding the surprise scale (2/D)*lr_s into T's s-columns lets
            # gg3 = v - pred with no broadcast dependency.
            stage = big.tile([P, 2], FP32)
            nc.vector.scalar_tensor_tensor(
                out=stage[:, 0:1], in0=nacc[:, 0:1],
                scalar=float(np.log(2.0 / D)), in1=lg[:, 2:3],
                op0=ALU.add, op1=ALU.add)
            nc.vector.tensor_copy(stage[:, 1:2], cacc[:, 1:2])
            scr = dram.tile([P, 2], FP32)
            nc.sync.dma_start(scr, stage)
            bc3 = big.tile([P, 2, N], FP32)
            for k in range(2):
                nc.sync.dma_start(bc3[:, k, :], bass.AP(
                    tensor=scr.tensor, offset=scr.offset + k,
                    ap=[[0, P], [2, N]]))
            ncarow = bc3[:, 0, :]
            pcdrow = bc3[:, 1, :]

            la = big.tile([P, N], FP32R)
            tmp1 = big.tile([P, N], FP32)
            nc.vector.scalar_tensor_tensor(
                out=tmp1, in0=ncarow, scalar=cacc[:, 0:1], in1=mls_sb,
                op0=ALU.add, op1=ALU.add)
            nc.scalar.activation(la, tmp1, AF.Exp)
            ldt = big.tile([P, N], FP32R)
            tmp2 = big.tile([P, N], FP32)
            nc.vector.scalar_tensor_tensor(
                out=tmp2, in0=pcdrow, scalar=nacc[:, 1:2], in1=mut_sb,
                op0=ALU.add, op1=ALU.add)
            nc.scalar.activation(ldt, tmp2, AF.Exp)

            tt_p = ptr.tile([N, N], FP32, tag="tr")
            nc.tensor.matmul(tt_p, ldt, la, start=True, stop=True)
            ttile = big.tile([P, N], FP32)
            nc.vector.memset(ttile, 0.0)
            nc.vector.tensor_copy(ttile[:N], tt_p)

            # maskbx_k[j, r] = T[toff + r//16, s(j)]  (expanded x16 in r)
            maskbx = []
            for k in range(8):
                ttx = rot.tile([P, P], FP32R, tag="ttx", bufs=2)
                nc.gpsimd.tensor_copy(
                    ttx[:N],
                    ttile[:N, k * 8:(k + 1) * 8, None].to_broadcast([N, 8, C]))
                mb_p = ptr.tile([P, C], FP32, tag="tr")
                nc.tensor.matmul(mb_p, ttx[:N], sel_sb[:N], start=True,
                                 stop=True)
                mb = rot.tile([P, C], FP32, tag="mb", bufs=2)
                nc.vector.tensor_copy(mb, mb_p)
                mbx = big.tile([P, RT], FP32, name=f"maskbx{k}")
                nc.gpsimd.tensor_copy(
                    mbx.rearrange("p (n c) -> p n c", c=C),
                    mb[:, :, None].to_broadcast([P, C, C]))
                maskbx.append(mbx)

            # ---------------- kv projection ----------------
            kT = [big.tile([P, L], FP32R, name=f"kT{k}") for k in range(2)]
            vT = [big.tile([P, L], FP32, name=f"vT{k}") for k in range(2)]
            for ko4 in range(4):
                dest = kT[ko4] if ko4 < 2 else vT[ko4 - 2]
                for rc in range(2):
                    sl = slice(rc * 512, (rc + 1) * 512)
                    mm = pmm.tile([P, 512], FP32, tag="mm")
                    for ki in range(2):
                        nc.tensor.matmul(
                            mm, wkv_sb[:, ki, ko4 * P:(ko4 + 1) * P],
                            snT[ki][:, sl], start=(ki == 0), stop=(ki == 1))
                    nc.vector.tensor_copy(dest[:, sl], mm)

            # ---------------- forward MLP ----------------
            Lf = [kT]
            dsT = []
            for i in range(3):
                a_next = [big.tile([P, L], FP32R, name=f"aT{i+1}_{k}")
                          for k in range(2)]
                ds_i = [big.tile([P, L], FP32, name=f"dsT{i}_{k}")
                        for k in range(2)]
                for mo in range(2):
                    for rc in range(2):
                        sl = slice(rc * 512, (rc + 1) * 512)
                        mm = pmm.tile([P, 512], FP32, tag="mm")
                        for ki in range(2):
                            nc.tensor.matmul(
                                mm, w_sb[:, i, ki, mo * P:(mo + 1) * P],
                                Lf[i][ki][:, sl],
                                start=(ki == 0), stop=(ki == 1))
                        sgt = rot.tile([P, 512], FP32, tag="sgt", bufs=2)
                        nc.scalar.activation(sgt, mm, AF.Sigmoid)
                        nc.vector.tensor_mul(a_next[mo][:, sl], mm, sgt)
                        # ds = sig * (1 + h - a); final mult off-path on gpsimd
                        t2 = rot.tile([P, 512], FP32, tag="t2", bufs=2)
                        nc.vector.scalar_tensor_tensor(
                            out=t2, in0=mm, scalar=1.0, in1=a_next[mo][:, sl],
                            op0=ALU.add, op1=ALU.subtract)
                        nc.gpsimd.tensor_mul(ds_i[mo][:, sl], sgt, t2)
                Lf.append(a_next)
                dsT.append(ds_i)

            # ---------------- pred + gg3 ----------------
            ggA = [big.tile([P, L], FP32R, name=f"ggA{k}", tag=f"snT{k}")
                   for k in range(2)]
            ggB = [big.tile([P, L], FP32R, name="ggB0", tag="sq8"),
                   big.tile([P, L], FP32R, name="ggB1", tag="qs2")]
            for mo in range(2):
                for rc in range(2):
                    sl = slice(rc * 512, (rc + 1) * 512)
                    mm = pmm.tile([P, 512], FP32, tag="mm")
                    for ki in range(2):
                        nc.tensor.matmul(
                            mm, w_sb[:, 3, ki, mo * P:(mo + 1) * P],
                            Lf[3][ki][:, sl], start=(ki == 0), stop=(ki == 1))
                    nc.vector.tensor_sub(ggA[mo][:, sl], vT[mo][:, sl], mm)

            # ---------------- R factors + backward ----------------
            Rf = {i: [big.tile([P, D], FP32R, name=f"Rf{i}_{jt}")
                      for jt in range(8)] for i in range(4)}

            def emit_R(layer, src):
                for jt in range(8):
                    tp = ptr.tile([P, 512], FP32R, tag="tr")
                    for mo in range(2):
                        nc.tensor.transpose(
                            tp[:, mo * P:(mo + 1) * P],
                            src[mo][:, jt * P:(jt + 1) * P], identR)
                    nc.vector.tensor_copy(Rf[layer][jt], tp[:, 0:D])

            emit_R(3, ggA)
            gg_cur, gg_next = ggA, ggB
            for i in (3, 2, 1):
                for mo in range(2):
                    for rc in range(2):
                        sl = slice(rc * 512, (rc + 1) * 512)
                        mm = pmm.tile([P, 512], FP32, tag="mm")
                        for ki in range(2):
                            nc.tensor.matmul(
                                mm, wt_sb[:, i - 1, ki, mo * P:(mo + 1) * P],
                                gg_cur[ki][:, sl],
                                start=(ki == 0), stop=(ki == 1))
                        nc.vector.tensor_mul(
                            gg_next[mo][:, sl], mm, dsT[i - 1][mo][:, sl])
                emit_R(i - 1, gg_next)
                gg_cur, gg_next = gg_next, gg_cur

            # ---------------- retrieve ----------------
            XTa = [big.tile([P, RT], FP32R, name=f"XTa{k}") for k in range(2)]
            XTb = [big.tile([P, RT], FP32R, name=f"XTb{k}") for k in range(2)]
            for mo in range(2):
                sc = psc.tile([P, RT], FP32, tag="sc")
                for ki in range(2):
                    nc.tensor.matmul(sc, wq_sb[:, ki, mo * P:(mo + 1) * P],
                                     rqT[ki], start=(ki == 0), stop=(ki == 1))
                nc.vector.tensor_copy(XTa[mo], sc)

            XTin, XTout = XTa, XTb
            X4T = [big.tile([P, RT], FP32R, name=f"X4T{k}") for k in range(2)]
            for i in range(4):
                msc = []
                for jt in range(8):
                    sc = psc.tile([P, RT], FP32, tag="sc")
                    for ki in range(2):
                        nc.tensor.matmul(
                            sc, Lf[i][ki][:, jt * P:(jt + 1) * P], XTin[ki],
                            start=(ki == 0), stop=(ki == 1))
                    m = rot.tile([P, RT], FP32R, tag="msc", bufs=8)
                    nc.vector.tensor_mul(m, sc, maskbx[jt])
                    msc.append(m)
                for mo in range(2):
                    y = psc.tile([P, RT], FP32, tag="y")
                    for ki in range(2):
                        nc.tensor.matmul(
                            y, w_sb[:, i, ki, mo * P:(mo + 1) * P], XTin[ki],
                            start=(ki == 0), stop=False)
                    for jt in range(8):
                        nc.tensor.matmul(
                            y, Rf[i][jt][:, mo * P:(mo + 1) * P], msc[jt],
                            start=False, stop=(jt == 7))
                    if i < 3:
                        sgt = rot.tile([P, RT], FP32, tag="sgr")
                        nc.scalar.activation(sgt, y, AF.Sigmoid)
                        nc.vector.tensor_mul(XTout[mo], y, sgt)
                    else:
                        nc.vector.tensor_copy(X4T[mo], y)
                XTin, XTout = XTout, XTin

            # ---------------- postnorm + output ----------------
            for rt in range(2):
                tp = ptr.tile([P, 512], FP32R, tag="tr")
                for mo in range(2):
                    nc.tensor.transpose(
                        tp[:, mo * P:(mo + 1) * P],
                        X4T[mo][:, rt * P:(rt + 1) * P], identR)
                x4 = rot.tile([P, D], FP32, tag="x4", bufs=2)
                nc.vector.tensor_copy(x4, tp[:, 0:D])
                scr_a = rot.tile([P, D], FP32, tag="rms_scr", bufs=2)
                ms = rot.tile([P, 1], FP32, tag="pms", bufs=2)
                nc.scalar.activation(scr_a, x4, AF.Square, accum_out=ms)
                lnv = rot.tile([P, 1], FP32, tag="pln", bufs=2)
                nc.scalar.activation(lnv, ms, AF.Ln, scale=1.0 / D, bias=eps_sb)
                rstd = rot.tile([P, 1], FP32, tag="prs", bufs=2)
                nc.scalar.activation(rstd, lnv, AF.Exp, scale=-0.5)
                o = rot.tile([P, D], FP32, tag="osb", bufs=2)
                nc.vector.tensor_scalar_mul(o, x4, rstd)
                nc.sync.dma_start(out_d[rt * P:(rt + 1) * P, :], o)

    nc.compile()
    return nc


def _host_prep(inputs):
    seq = np.ascontiguousarray(np.asarray(inputs["seq"], dtype=np.float32))
    Wq = np.asarray(inputs["Wq"], dtype=np.float32)
    Wkv = np.asarray(inputs["Wkv"], dtype=np.float32)
    Ws = [np.asarray(inputs[f"W{i}"], dtype=np.float32) for i in range(4)]
    wa = np.asarray(inputs["w_adapt"], dtype=np.float32)
    wm = np.asarray(inputs["w_mom"], dtype=np.float32)
    wd = np.asarray(inputs["w_decay"], dtype=np.float32)

    def kxm(w):  # [K, M] -> [128, (K/128)*M]
        return w.reshape(w.shape[0] // P, P, w.shape[1]).transpose(1, 0, 2) \
            .reshape(P, -1)

    ii = np.arange(N)
    tri = np.triu(np.ones((N, N), np.float32))
    wpack = np.zeros((D, 4), np.float32)
    wpack[:, 0] = wa
    wpack[:, 1] = wm
    wpack[:, 2] = wd
    wpack *= (1.0 / C)

    wts = np.zeros((P, WTS_SZ), np.float32)
    wts[:, WQ_O:WQ_O + 512] = kxm(Wq)
    wts[:, WKV_O:WKV_O + 1024] = kxm(Wkv)
    w_all = np.stack(Ws).reshape(4, 2, P, D).transpose(2, 0, 1, 3)
    wts[:, W_O:W_O + 2048] = w_all.reshape(P, -1)
    wt_all = np.stack([Ws[1].T, Ws[2].T, Ws[3].T]) \
        .reshape(3, 2, P, D).transpose(2, 0, 1, 3)
    wts[:, WT_O:WT_O + 1536] = wt_all.reshape(P, -1)
    wts[:, WP_O:WP_O + 8] = kxm(wpack)
    wts[:, IDR_O:IDR_O + 128] = np.eye(P, dtype=np.float32)
    wts[:N, UT_O:UT_O + N] = tri
    wts[:N, NUT_O:NUT_O + N] = -tri

    cst = np.full((P, 2 * N), -1e30, np.float32)
    cst[:N, 0:N] = np.where(ii[:, None] >= ii[None, :], 0.0, -1e30)
    cst[:N, N:2 * N] = np.where(ii[:, None] <= ii[None, :], 0.0, -1e30)

    in_maps = []
    for core in range(NCORES):
        b, g = divmod(core, GROUPS)
        wts_c = wts.copy()
        sel = np.zeros((P, C), np.float32)
        toff = C * g
        sel[toff:toff + C, :] = np.eye(C, dtype=np.float32)
        wts_c[:, SEL_O:SEL_O + C] = sel
        m = {"wts_d": wts_c, "cst_d": cst, "seq_b": seq[b]}
        qs = np.zeros((RT, D), np.float32)
        j0 = RT * g + (C - 1)
        src = seq[b, j0:min(j0 + RT, L)]
        qs[:len(src)] = src
        m["seq_q"] = qs
        in_maps.append(m)
    return in_maps


def kernel(**inputs):
    if "nc" not in _CACHE:
        _CACHE["nc"] = _build()
    nc = _CACHE["nc"]
    in_maps = _host_prep(inputs)
    trace = bool(int(os.environ.get("KERNEL_TRACE", "0")))
    if trace:
        try:
            from antenv.axon_hooks import get_axon_ntff_profile_hook  # noqa: F401
        except ImportError:
            trace = _install_ntff_hook()
    res = run_bass_kernel_spmd(
        nc, in_maps, core_ids=list(range(NCORES)), trace=trace)
    LAST_PERF.clear()
    LAST_PERF.update(dict(
        exec_time_ns=res.exec_time_ns,
        mean_exec_time_ns=res.mean_exec_time_ns,
        profile_json=res.profile_json,
        trace=res.instructions_and_trace[1] if res.instructions_and_trace else None,
    ))
    final = np.zeros((B, L, D), np.float32)
    for core in range(NCORES):
        b, g = divmod(core, GROUPS)
        j0 = RT * g + (C - 1)
        n = min(RT, L - j0)
        final[b, j0:j0 + n] = res.results[core]["out"][:n]
    return final



# revision 43
# speedup vs baseline: 1.6904x; 1.6904x over previous
"""Titans NeuralMemory forward on 8 Trainium2 NeuronCores.

Decomposition (validated vs reference in fp64/numpy):
  - Per-chunk MLP-loss gradients are rank-16: g_i(s) = l_i(s)^T r_i(s) with
    l/r factors [16, 256] from a batched forward/backward pass with the
    shared base weights.
  - The two associative scans have scalar per-chunk coefficients, so their
    composition is a lower-triangular [64, 64] matrix T = L_D @ L_A built
    stably via exp of cumulative log-sigmoid differences (softplus form).
  - Retrieval never materializes fast weights: per layer,
      X_{i+1} = silu(X_i @ W_i + (X_i @ L_i^T * M) @ R_i),
    where M[r, j] = T[chunk(r), chunk(j)] expands T blockwise.

Sharding: 8 cores = 2 batch rows x 4 retrieve row-groups of 256 rows.
Each core redundantly runs the store phase for its batch row and computes
its own 256 retrieve rows; no collectives. Matmuls in fp32r (full PE rate).

v2 vs v1: on-chip T-row broadcast (no DRAM round trip), activation ops
grouped by table set (Rsqrt/Softplus/Sigmoid/Exp), masks via PE matmuls
instead of gpsimd broadcasts, DMA split across engine queues by first use,
PE warm-up burst, emission order tuned against queue head-of-line blocks.
"""
import os
import numpy as np

import concourse.bass as bass
import concourse.tile as tile
from concourse import bacc, mybir
from concourse.bass_utils import run_bass_kernel_spmd

AF = mybir.ActivationFunctionType
ALU = mybir.AluOpType
FP32 = mybir.dt.float32
FP32R = mybir.dt.float32r

B, L, D, C, DEPTH = 2, 1024, 256, 16, 4
N = L // C          # 64 chunks
P = 128
EPS = 1.1920929e-07
NCORES = 8
GROUPS = 4
RT = L // GROUPS    # 256 retrieve rows per core
LN2D = float(np.log(2.0 / D))

# wts_e layout (per-partition fp32 word offsets)
IDR_O = 0            # identity [128,128]
EX8_O = 128          # ex8 [8,128]: ex8[a,p] = (p//16 == a)
SELX_O = 256         # selx [64,256]: selx[n,r] = (n == toff + r//16)
UT_O = 512           # tri  [64,64] upper-triangular ones
NUT_O = 576          # -tri [64,64]
WP_O = 640           # wp [128,2,4] (adapt/mom/decay/0, /C, kxm layout)
ONES_O = 648         # row selectors [2,64]: cols 0:64 pick row0, 64:128 row1
E_SZ = 776

_CACHE = {}
LAST_PERF = {}


def _install_ntff_hook():
    import sys
    import types
    try:
        from trn_agent_boot.trn_boot import _ntff_profile_via_ctypes
        hook = _ntff_profile_via_ctypes("/opt/axon/libaxon_pjrt.so")
    except Exception:
        return False
    if hook is None:
        return False
    mod = types.ModuleType("antenv.axon_hooks")
    mod.get_axon_ntff_profile_hook = lambda: hook
    mod.set_axon_ntff_profile_hook = lambda h: None
    sys.modules["antenv.axon_hooks"] = mod
    return True


def _build():
    nc = bacc.Bacc("TRN2", target_bir_lowering=False)

    seq_b = nc.dram_tensor("seq_b", [L, D], FP32, kind="ExternalInput")
    seq_q = nc.dram_tensor("seq_q", [RT, D], FP32, kind="ExternalInput")
    wts_e_d = nc.dram_tensor("wts_e", [P, E_SZ], FP32R, kind="ExternalInput")
    wts_qkv_d = nc.dram_tensor("wts_qkv", [P, 1536], FP32R,
                               kind="ExternalInput")
    wts_w_d = nc.dram_tensor("wts_w", [P, 2048], FP32R, kind="ExternalInput")
    wts_wt_d = nc.dram_tensor("wts_wt", [P, 1536], FP32R,
                              kind="ExternalInput")
    cst_d = nc.dram_tensor("cst_d", [P, 2 * N], FP32, kind="ExternalInput")
    out_d = nc.dram_tensor("out", [RT, D], FP32, kind="ExternalOutput")

    with tile.TileContext(nc) as tc:
        with (
            tc.tile_pool(name="big", bufs=1) as big,
            tc.tile_pool(name="rot", bufs=3) as rot,
            tc.tile_pool(name="pmm", bufs=2, space="PSUM") as pmm,
            tc.tile_pool(name="psc", bufs=2, space="PSUM") as psc,
            tc.tile_pool(name="ptr", bufs=2, space="PSUM") as ptr,
        ):
            # ---------------- DMA: split by first use across queues ------
            sq8 = big.tile([P, 8, D], FP32, tag="sq8")
            nc.sync.dma_start(sq8, seq_b[:].rearrange("(i p) d -> p i d", p=P))
            wts_e = big.tile([P, E_SZ], FP32R)
            nc.scalar.dma_start(wts_e, wts_e_d[:])
            wts_qkv = big.tile([P, 1536], FP32R)
            nc.gpsimd.dma_start(wts_qkv, wts_qkv_d[:])
            wts_w = big.tile([P, 2048], FP32R)
            nc.gpsimd.dma_start(wts_w, wts_w_d[:])
            qs2 = big.tile([P, 2, D], FP32, tag="qs2")
            nc.sync.dma_start(qs2, seq_q[:].rearrange("(i p) d -> p i d", p=P))
            cst = big.tile([P, 2 * N], FP32)
            nc.sync.dma_start(cst, cst_d[:])
            wts_wt = big.tile([P, 1536], FP32R)
            nc.gpsimd.dma_start(wts_wt, wts_wt_d[:])

            identR = wts_e[:, IDR_O:IDR_O + 128]
            ex8 = wts_e[:, EX8_O:EX8_O + 128]
            selx = wts_e[:, SELX_O:SELX_O + 256]
            tri_r = wts_e[:, UT_O:UT_O + N]
            ntri_r = wts_e[:, NUT_O:NUT_O + N]
            wp_sb = wts_e[:, WP_O:WP_O + 8].rearrange("p (k m) -> p k m", k=2)
            wq_sb = wts_qkv[:, 0:512].rearrange("p (k m) -> p k m", k=2)
            wkv_sb = wts_qkv[:, 512:1536].rearrange("p (k m) -> p k m", k=2)
            w_sb = wts_w[:].rearrange("p (l k m) -> p l k m", l=4, k=2)
            wt_sb = wts_wt[:].rearrange("p (l k m) -> p l k m", l=3, k=2)
            mls_sb = cst[:, 0:N]
            mut_sb = cst[:, N:2 * N]

            eps_sb = big.tile([P, 1], FP32)
            nc.vector.memset(eps_sb, EPS)
            selr0 = wts_e[0:2, ONES_O:ONES_O + N]
            selr1 = wts_e[0:2, ONES_O + N:ONES_O + 2 * N]

            # ---------------- PE warm-up (defeat HAM cold clock) ---------
            for _ in range(12):
                wp_ps = pmm.tile([P, 512], FP32, tag="mm")
                nc.tensor.matmul(wp_ps, identR, wts_e[:, 0:512],
                                 start=True, stop=True)

            # ---------------- rmsnorms, grouped by ACT table ------------
            # square is in every table set; rsqrt loads its set once.
            xs = [sq8[:, i, :] for i in range(8)] + [qs2[:, i, :]
                                                    for i in range(2)]
            mss, rstds = [], []
            for i, x in enumerate(xs):
                scr_a = rot.tile([P, D], FP32, tag="rms_scr", bufs=2)
                ms = rot.tile([P, 1], FP32, tag="rms_ms", bufs=10)
                nc.scalar.activation(scr_a, x, AF.Square, accum_out=ms)
                mss.append(ms)
            lnvs = []
            for i, ms in enumerate(mss):
                lnv = rot.tile([P, 1], FP32, tag="rms_ln", bufs=10)
                nc.scalar.activation(lnv, ms, AF.Ln, scale=1.0 / D,
                                     bias=eps_sb)
                lnvs.append(lnv)
            for i, lnv in enumerate(lnvs):
                rstd = rot.tile([P, 1], FP32, tag="rms_rs", bufs=10)
                nc.scalar.activation(rstd, lnv, AF.Exp, scale=-0.5)
                rstds.append(rstd)
            sn, rq = [], []
            for i, x in enumerate(xs):
                o = rot.tile([P, D], FP32R, tag=f"rms_o{i}", bufs=1)
                nc.vector.tensor_scalar_mul(o, x, rstds[i])
                (sn if i < 8 else rq).append(o)

            # ---------------- transposes: snT, rqT ----------------------
            snT = [big.tile([P, L], FP32R, name=f"snT{k}", tag=f"snT{k}")
                   for k in range(2)]
            for grp in range(2):
                for ko in range(2):
                    tp = ptr.tile([P, 512], FP32R, tag="tr")
                    for ii in range(4):
                        i = grp * 4 + ii
                        nc.tensor.transpose(
                            tp[:, ii * P:(ii + 1) * P],
                            sn[i][:, ko * P:(ko + 1) * P], identR)
                    nc.vector.tensor_copy(
                        snT[ko][:, grp * 512:(grp + 1) * 512], tp)
            rqT = [big.tile([P, RT], FP32R, name=f"rqT{k}") for k in range(2)]
            for ko in range(2):
                tp = ptr.tile([P, 512], FP32R, tag="tr")
                for rt in range(2):
                    nc.tensor.transpose(
                        tp[:, rt * P:(rt + 1) * P],
                        rq[rt][:, ko * P:(ko + 1) * P], identR)
                nc.vector.tensor_copy(rqT[ko], tp[:, 0:RT])

            # ---------------- chunk sums -> zp / zpT --------------------
            cmT = big.tile([P, 2, N], FP32R)
            with nc.allow_low_precision(reason="fp32r rounding of fp32 accum"):
                for ko in range(2):
                    nc.vector.reduce_sum(
                        cmT[:, ko, :],
                        snT[ko].rearrange("p (n c) -> p n c", c=C),
                        axis=mybir.AxisListType.X)

            # one PSUM bank carved into all the small T-pipeline outputs
            tpipe = ptr.tile([P, 512], FP32, tag="tpipe", bufs=1)
            zp = tpipe[0:N, 0:4]
            zpT = tpipe[0:4, 4:68]
            rowp2 = tpipe[0:2, 68:196]
            nsps = tpipe[0:N, 196:198]
            bcA = tpipe[0:N, 198:262]
            bcB = tpipe[0:N, 262:326]
            tt_p = tpipe[0:N, 326:390]
            for ko in range(2):
                nc.tensor.matmul(zp, cmT[:, ko, :], wp_sb[:, ko, :],
                                 start=(ko == 0), stop=(ko == 1))
            for ko in range(2):
                nc.tensor.matmul(zpT, wp_sb[:, ko, :], cmT[:, ko, :],
                                 start=(ko == 0), stop=(ko == 1))

            # sigmoids now (also preloads the sigmoid table for the fwd MLP);
            # the ln/exp cluster is deferred into the backward window.
            sgc = big.tile([N, 2], FP32)
            nc.scalar.activation(sgc[:, 0:1], zp[:, 1:2], AF.Sigmoid)
            nc.scalar.activation(sgc[:, 1:2], zp[:, 2:3], AF.Sigmoid,
                                 scale=-1.0)
            sgLT = big.tile([2, N], FP32)
            nc.scalar.activation(sgLT, zpT[0:2, :], AF.Sigmoid)

            # ---------------- kv projection -----------------------------
            kT = [big.tile([P, L], FP32R, name=f"kT{k}") for k in range(2)]
            vT = [big.tile([P, L], FP32, name=f"vT{k}") for k in range(2)]
            for ko4 in range(4):
                dest = kT[ko4] if ko4 < 2 else vT[ko4 - 2]
                for rc in range(2):
                    sl = slice(rc * 512, (rc + 1) * 512)
                    mm = pmm.tile([P, 512], FP32, tag="mm")
                    for ki in range(2):
                        nc.tensor.matmul(
                            mm, wkv_sb[:, ki, ko4 * P:(ko4 + 1) * P],
                            snT[ki][:, sl], start=(ki == 0), stop=(ki == 1))
                    if ko4 < 2:
                        nc.vector.tensor_copy(dest[:, sl], mm)
                    else:
                        nc.scalar.activation(dest[:, sl], mm, AF.Copy)

            # ---------------- q projection (early; needed at retrieve) --
            XTa = [big.tile([P, RT], FP32R, name=f"XTa{k}") for k in range(2)]
            XTb = [big.tile([P, RT], FP32R, name=f"XTb{k}") for k in range(2)]
            for mo in range(2):
                sc = psc.tile([P, RT], FP32, tag="y")
                for ki in range(2):
                    nc.tensor.matmul(sc, wq_sb[:, ki, mo * P:(mo + 1) * P],
                                     rqT[ki], start=(ki == 0), stop=(ki == 1))
                nc.vector.tensor_copy(XTa[mo], sc)

            # ---------------- forward MLP -------------------------------
            Lf = [kT]
            dsT = []
            for i in range(3):
                a_next = [big.tile([P, L], FP32R, name=f"aT{i+1}_{k}")
                          for k in range(2)]
                ds_i = [big.tile([P, L], FP32, name=f"dsT{i}_{k}")
                        for k in range(2)]
                for mo in range(2):
                    for rc in range(2):
                        sl = slice(rc * 512, (rc + 1) * 512)
                        mm = pmm.tile([P, 512], FP32, tag="mm")
                        for ki in range(2):
                            nc.tensor.matmul(
                                mm, w_sb[:, i, ki, mo * P:(mo + 1) * P],
                                Lf[i][ki][:, sl],
                                start=(ki == 0), stop=(ki == 1))
                        sgt = rot.tile([P, 512], FP32, tag="sgt", bufs=2)
                        nc.scalar.activation(sgt, mm, AF.Sigmoid)
                        nc.vector.tensor_mul(a_next[mo][:, sl], mm, sgt)
                        # ds = sig * (1 + h - a); mult off-path
                        t2 = rot.tile([P, 512], FP32, tag="t2", bufs=2)
                        nc.vector.scalar_tensor_tensor(
                            out=t2, in0=mm, scalar=1.0, in1=a_next[mo][:, sl],
                            op0=ALU.add, op1=ALU.subtract)
                        if i < 2:
                            nc.gpsimd.tensor_mul(ds_i[mo][:, sl], sgt, t2)
                        else:
                            nc.vector.tensor_mul(ds_i[mo][:, sl], sgt, t2)
                Lf.append(a_next)
                dsT.append(ds_i)

            # ---------------- pred + gg3 --------------------------------
            ggA = [big.tile([P, L], FP32R, name=f"ggA{k}", tag=f"snT{k}")
                   for k in range(2)]
            ggB = [big.tile([P, L], FP32R, name="ggB0", tag="sq8"),
                   big.tile([P, L], FP32R, name="ggB1", tag="qs2")]
            for mo in range(2):
                for rc in range(2):
                    sl = slice(rc * 512, (rc + 1) * 512)
                    mm = pmm.tile([P, 512], FP32, tag="mm")
                    for ki in range(2):
                        nc.tensor.matmul(
                            mm, w_sb[:, 3, ki, mo * P:(mo + 1) * P],
                            Lf[3][ki][:, sl], start=(ki == 0), stop=(ki == 1))
                    nc.vector.tensor_sub(ggA[mo][:, sl], vT[mo][:, sl], mm)

            # ---------------- R factors + backward ----------------------
            Rf = {i: [big.tile([P, D], FP32R, name=f"Rf{i}_{jt}")
                      for jt in range(8)] for i in range(4)}

            def emit_R(layer, src, engines):
                for jt in range(8):
                    tp = ptr.tile([P, 512], FP32R, tag="tr")
                    for mo in range(2):
                        nc.tensor.transpose(
                            tp[:, mo * P:(mo + 1) * P],
                            src[mo][:, jt * P:(jt + 1) * P], identR)
                    eng = engines[jt % len(engines)]
                    if eng == "s":
                        nc.scalar.activation(
                            Rf[layer][jt],
                            tp[:, 0:D].bitcast(FP32), AF.Copy)
                    else:
                        nc.vector.tensor_copy(Rf[layer][jt], tp[:, 0:D])

            emit_R(3, ggA, ("v", "s"))
            gg_cur, gg_next = ggA, ggB
            for i in (3, 2, 1):
                for mo in range(2):
                    for rc in range(2):
                        sl = slice(rc * 512, (rc + 1) * 512)
                        mm = pmm.tile([P, 512], FP32, tag="mm")
                        for ki in range(2):
                            nc.tensor.matmul(
                                mm, wt_sb[:, i - 1, ki, mo * P:(mo + 1) * P],
                                gg_cur[ki][:, sl],
                                start=(ki == 0), stop=(ki == 1))
                        nc.vector.tensor_mul(
                            gg_next[mo][:, sl], mm, dsT[i - 1][mo][:, sl])
                emit_R(i - 1, gg_next,
                       ("v", "s") if i > 1 else ("v",))
                gg_cur, gg_next = gg_next, gg_cur

            # ---------------- T pipeline (runs in the bwd window) -------
            # ln/exp cluster deferred here: natural_log_exp loads while the
            # PE chews on backward matmuls; masks only needed at retrieve.
            lgc = big.tile([N, 2], FP32R)
            nc.scalar.activation(lgc, sgc, AF.Ln)
            lgT = big.tile([2, N], FP32)
            nc.scalar.activation(lgT, sgLT, AF.Ln)
            # rowp[0:2, 0:N] = -[cumsum lg0; cumsum lg1]^T,
            # rowp[0:2, N:2N] = +[cumsum lg0; cumsum lg1]^T
            nc.tensor.matmul(rowp2[:, 0:N], lgc, ntri_r[:N, :],
                             start=True, stop=True)
            nc.tensor.matmul(rowp2[:, N:2 * N], lgc, tri_r[:N, :],
                             start=True, stop=True)
            # nsps[:, 0:2] = [cumsum lg0, cumsum lg1] per chunk row
            nc.tensor.matmul(nsps, tri_r[:N, :N], lgc, start=True, stop=True)
            nsps_sb = big.tile([N, 2], FP32)
            nc.vector.tensor_copy(nsps_sb, nsps)
            # rowA row0 = -cumsum(lg0)[j] + ln(2/D) + lglr[j].  Keep fp32:
            # cumsums reach +-45 and fp32r rounding there costs ~2% after exp.
            rowA = big.tile([2, N], FP32)
            nc.vector.scalar_tensor_tensor(
                out=rowA, in0=rowp2[:, 0:N], scalar=LN2D, in1=lgT,
                op0=ALU.add, op1=ALU.add)
            # rowB row1 = +cumsum(lg1)[j]
            rowB = big.tile([2, N], FP32)
            nc.vector.tensor_copy(rowB, rowp2[:, N:2 * N])
            nc.tensor.matmul(bcA, selr0.bitcast(FP32), rowA,
                             start=True, stop=True)
            nc.tensor.matmul(bcB, selr1.bitcast(FP32), rowB,
                             start=True, stop=True)
            tmp1 = big.tile([N, N], FP32)
            nc.vector.scalar_tensor_tensor(
                out=tmp1, in0=bcA, scalar=nsps_sb[:, 0:1], in1=mls_sb[:N, :],
                op0=ALU.add, op1=ALU.add)
            tmp2 = big.tile([N, N], FP32)
            nc.vector.scalar_tensor_tensor(
                out=tmp2, in0=bcB, scalar=nsps_sb[:, 1:2], in1=mut_sb[:N, :],
                op0=ALU.subtract, op1=ALU.add)
            la = big.tile([N, N], FP32R)
            nc.scalar.activation(la, tmp1, AF.Exp)
            ldt = big.tile([N, N], FP32R)
            nc.scalar.activation(ldt, tmp2, AF.Exp)

            # ---------------- T matrix + masks (PE, not gpsimd) ---------
            nc.tensor.matmul(tt_p, ldt, la, start=True, stop=True)
            ttile = big.tile([N, N], FP32R)
            nc.vector.tensor_copy(ttile, tt_p)
            # maskbx[jt][p, r] = T[toff + r//16, jt*8 + p//16]
            maskbx = []
            for jt in range(8):
                o1f = psc.tile([P, RT], FP32, tag="y")
                o1p = o1f[0:8, :]
                nc.tensor.matmul(o1p, ttile[:, jt * 8:(jt + 1) * 8],
                                 selx[:N, :], start=True, stop=True)
                o1 = rot.tile([8, RT], FP32R, tag="o1s", bufs=2)
                nc.vector.tensor_copy(o1, o1p)
                mbf = pmm.tile([P, 512], FP32, tag="mm", name="mbf")
                mbp = mbf[:, 0:RT]
                nc.tensor.matmul(mbp, ex8[0:8, :], o1, start=True, stop=True)
                mbx = big.tile([P, RT], FP32, name=f"maskbx{jt}")
                if jt % 2:
                    nc.scalar.activation(mbx, mbp, AF.Copy)
                else:
                    nc.vector.tensor_copy(mbx, mbp)
                maskbx.append(mbx)

            # ---------------- retrieve ----------------------------------
            XTin, XTout = XTa, XTb
            X4T = [big.tile([P, RT], FP32R, name=f"X4T{k}") for k in range(2)]
            for i in range(4):
                msc = []
                for jt in range(8):
                    scf = pmm.tile([P, 512], FP32, tag="mm", name="scf")
                    sc = scf[:, 0:RT]
                    for ki in range(2):
                        nc.tensor.matmul(
                            sc, Lf[i][ki][:, jt * P:(jt + 1) * P], XTin[ki],
                            start=(ki == 0), stop=(ki == 1))
                    m = rot.tile([P, RT], FP32R, tag="msc", bufs=8)
                    nc.vector.tensor_mul(m, sc, maskbx[jt])
                    msc.append(m)
                for mo in range(2):
                    y = psc.tile([P, RT], FP32, tag="y")
                    for ki in range(2):
                        nc.tensor.matmul(
                            y, w_sb[:, i, ki, mo * P:(mo + 1) * P], XTin[ki],
                            start=(ki == 0), stop=False)
                    for jt in range(8):
                        nc.tensor.matmul(
                            y, Rf[i][jt][:, mo * P:(mo + 1) * P], msc[jt],
                            start=False, stop=(jt == 7))
                    if i < 3:
                        sgt = rot.tile([P, RT], FP32, tag="sgr")
                        nc.scalar.activation(sgt, y, AF.Sigmoid)
                        nc.vector.tensor_mul(XTout[mo], y, sgt)
                    else:
                        nc.vector.tensor_copy(X4T[mo], y)
                XTin, XTout = XTout, XTin

            # ---------------- postnorm + output -------------------------
            for rt in range(2):
                tp = ptr.tile([P, 512], FP32R, tag="tr")
                for mo in range(2):
                    nc.tensor.transpose(
                        tp[:, mo * P:(mo + 1) * P],
                        X4T[mo][:, rt * P:(rt + 1) * P], identR)
                x4 = rot.tile([P, D], FP32, tag="x4", bufs=2)
                nc.vector.tensor_copy(x4, tp[:, 0:D])
                scr_a = rot.tile([P, D], FP32, tag="rms_scr", bufs=2)
                ms = rot.tile([P, 1], FP32, tag="pms", bufs=2)
                nc.scalar.activation(scr_a, x4, AF.Square, accum_out=ms)
                lnv = rot.tile([P, 1], FP32, tag="pln", bufs=2)
                nc.scalar.activation(lnv, ms, AF.Ln, scale=1.0 / D,
                                     bias=eps_sb)
                rstd = rot.tile([P, 1], FP32, tag="prs", bufs=2)
                nc.scalar.activation(rstd, lnv, AF.Exp, scale=-0.5)
                o = rot.tile([P, D], FP32, tag="osb", bufs=2)
                nc.vector.tensor_scalar_mul(o, x4, rstd)
                nc.sync.dma_start(out_d[rt * P:(rt + 1) * P, :], o)

    nc.compile()
    return nc


def _host_prep(inputs):
    seq = np.ascontiguousarray(np.asarray(inputs["seq"], dtype=np.float32))
    Wq = np.asarray(inputs["Wq"], dtype=np.float32)
    Wkv = np.asarray(inputs["Wkv"], dtype=np.float32)
    Ws = [np.asarray(inputs[f"W{i}"], dtype=np.float32) for i in range(4)]
    wa = np.asarray(inputs["w_adapt"], dtype=np.float32)
    wm = np.asarray(inputs["w_mom"], dtype=np.float32)
    wd = np.asarray(inputs["w_decay"], dtype=np.float32)

    def kxm(w):  # [K, M] -> [128, (K/128)*M]
        return w.reshape(w.shape[0] // P, P, w.shape[1]).transpose(1, 0, 2) \
            .reshape(P, -1)

    ii = np.arange(N)
    tri = np.triu(np.ones((N, N), np.float32))
    wpack = np.zeros((D, 4), np.float32)
    wpack[:, 0] = wa
    wpack[:, 1] = wm
    wpack[:, 2] = wd
    wpack *= (1.0 / C)

    wts_e = np.zeros((P, E_SZ), np.float32)
    wts_e[:, IDR_O:IDR_O + 128] = np.eye(P, dtype=np.float32)
    aa = np.arange(P)
    wts_e[0:8, EX8_O:EX8_O + 128] = (aa[None, :] // 16 ==
                                     np.arange(8)[:, None]).astype(np.float32)
    wts_e[:N, UT_O:UT_O + N] = tri
    wts_e[:N, NUT_O:NUT_O + N] = -tri
    wts_e[:, WP_O:WP_O + 8] = kxm(wpack)
    wts_e[0, ONES_O:ONES_O + N] = 1.0
    wts_e[1, ONES_O + N:ONES_O + 2 * N] = 1.0

    wts_qkv = np.zeros((P, 1536), np.float32)
    wts_qkv[:, 0:512] = kxm(Wq)
    wts_qkv[:, 512:1536] = kxm(Wkv)
    w_all = np.stack(Ws).reshape(4, 2, P, D).transpose(2, 0, 1, 3)
    wts_w = np.ascontiguousarray(w_all.reshape(P, -1))
    wt_all = np.stack([Ws[1].T, Ws[2].T, Ws[3].T]) \
        .reshape(3, 2, P, D).transpose(2, 0, 1, 3)
    wts_wt = np.ascontiguousarray(wt_all.reshape(P, -1))

    cst = np.full((P, 2 * N), -1e30, np.float32)
    cst[:N, 0:N] = np.where(ii[:, None] >= ii[None, :], 0.0, -1e30)
    cst[:N, N:2 * N] = np.where(ii[:, None] <= ii[None, :], 0.0, -1e30)

    rr = np.arange(RT)
    in_maps = []
    for core in range(NCORES):
        b, g = divmod(core, GROUPS)
        wts_e_c = wts_e.copy()
        toff = C * g
        wts_e_c[:N, SELX_O:SELX_O + RT] = (
            ii[:, None] == toff + rr[None, :] // 16).astype(np.float32)
        m = {"wts_e": wts_e_c, "wts_qkv": wts_qkv, "wts_w": wts_w,
             "wts_wt": wts_wt, "cst_d": cst, "seq_b": seq[b]}
        qs = np.zeros((RT, D), np.float32)
        j0 = RT * g + (C - 1)
        src = seq[b, j0:min(j0 + RT, L)]
        qs[:len(src)] = src
        m["seq_q"] = qs
        in_maps.append(m)
    return in_maps


def kernel(**inputs):
    if "nc" not in _CACHE:
        _CACHE["nc"] = _build()
    nc = _CACHE["nc"]
    in_maps = _host_prep(inputs)
    trace = bool(int(os.environ.get("KERNEL_TRACE", "0")))
    if trace:
        try:
            from antenv.axon_hooks import get_axon_ntff_profile_hook  # noqa: F401
        except ImportError:
            trace = _install_ntff_hook()
    res = run_bass_kernel_spmd(
        nc, in_maps, core_ids=list(range(NCORES)), trace=trace)
    LAST_PERF.clear()
    LAST_PERF.update(dict(
        exec_time_ns=res.exec_time_ns,
        mean_exec_time_ns=res.mean_exec_time_ns,
        profile_json=res.profile_json,
        trace=res.instructions_and_trace[1] if res.instructions_and_trace else None,
    ))
    final = np.zeros((B, L, D), np.float32)
    for core in range(NCORES):
        b, g = divmod(core, GROUPS)
        j0 = RT * g + (C - 1)
        n = min(RT, L - j0)
        final[b, j0:j0 + n] = res.results[core]["out"][:n]
    return final
